# revision 1
# baseline (speedup 1.0000x reference)
"""CKConv (SIREN continuous-kernel causal conv) Trainium2 Bass kernel.

Problem dims (hardcoded): B=32, CIN=32, COUT=32, T=2048, DK=32, K=T+1=2049.

Strategy: data-parallel over batch across 8 NeuronCores (4 samples/core).
Each core:
  1. runs the tiny SIREN kernel-net on-chip (fp32) to generate the conv
     kernel, laid out as KT[i][dtau, 32c+o] = kern[o,i,128c+dtau] (bf16),
     with a 17th column-block holding the single tap kern[o,i,2048] in
     row dtau=0,
  2. zero-pads its x shard into xe[b,i,:] = [0]*128 ++ x ++ [0]*256 (bf16,
     in DRAM),
  3. computes the causal conv as a sum of Hankel x Toeplitz block matmuls:
     for each input window a in 15..31, the stationary operand is the
     Hankel tile H[b,a,i][p,d] = xe[b,i,128*(a-15)+p+d] and the moving
     operand is a contiguous slice of KT packing all valid kernel chunks
     (columns (c,o)); PSUM accumulates over i; DVE scatters the per-c
     column blocks into per-(b,tb) output accumulators,
  4. adds the output bias and DMAs out[b,o,t] (transposed via strided DMA).

The host-side wrapper only reshapes/transposes weights (pure layout) and
concatenates per-core results.
"""
import os
import numpy as np

from contextlib import ExitStack

import concourse.bass as bass
import concourse.tile as tile
from concourse import bacc, mybir
from concourse.bass_utils import run_bass_kernel_spmd

F32 = mybir.dt.float32
BF16 = mybir.dt.bfloat16

B, CIN, COUT, T, DK = 32, 32, 32, 2048, 32
K = T + 1
L = 128
NB = T // L          # 16 time blocks
NCORES = 8
BSH = B // NCORES    # 4 batch samples per core
XE_LEN = 128 + T + 256  # 2432

_CACHED = {}


def _build(bsh: int = BSH, reps: int = 1, ablate: frozenset = frozenset()):
    """Build + schedule the per-core Bass program (SPMD, no collectives).

    reps>1 wraps the main conv loop in a dynamic For_i that repeats it —
    a timing rig to amortize dispatch overhead (outputs stay correct only
    for reps=1 ... actually they stay correct since out_sb is re-inited
    inside the loop).
    """
    nc = bacc.Bacc(
        "TRN2", target_bir_lowering=False, debug=False, enable_asserts=False
    )

    xh = nc.dram_tensor("x", [bsh, CIN, T], F32, kind="ExternalInput")
    rph = nc.dram_tensor("rel_pos", [K], F32, kind="ExternalInput")
    w1h = nc.dram_tensor("w1", [DK], F32, kind="ExternalInput")
    b1h = nc.dram_tensor("b1", [DK], F32, kind="ExternalInput")
    om1h = nc.dram_tensor("om1", [1], F32, kind="ExternalInput")
    w2th = nc.dram_tensor("w2t", [DK, DK], F32, kind="ExternalInput")
    b2h = nc.dram_tensor("b2", [DK], F32, kind="ExternalInput")
    om2h = nc.dram_tensor("om2", [1], F32, kind="ExternalInput")
    w3ah = nc.dram_tensor("w3a", [DK + 1, CIN * COUT], F32, kind="ExternalInput")
    biash = nc.dram_tensor("bias", [COUT], F32, kind="ExternalInput")
    outh = nc.dram_tensor("out", [bsh, COUT, T], F32, kind="ExternalOutput")

    xeh = nc.dram_tensor("xe", [bsh, CIN, XE_LEN], BF16)  # internal

    with tile.TileContext(nc) as tc, ExitStack() as ctx:
        singles = ctx.enter_context(tc.tile_pool(name="singles", bufs=1))
        hankp = ctx.enter_context(tc.tile_pool(name="hankp", bufs=3))
        kgps = ctx.enter_context(tc.tile_pool(name="kgps", bufs=2, space="PSUM"))
        mainps = ctx.enter_context(tc.tile_pool(name="mainps", bufs=6, space="PSUM"))

        # ---- small constants / broadcasts ----
        pos_b = singles.tile([DK, K], F32)
        nc.sync.dma_start(out=pos_b, in_=bass.AP(rph, 0, [[0, DK], [1, K]]))
        w1_sb = singles.tile([DK, 1], F32)
        nc.sync.dma_start(out=w1_sb, in_=bass.AP(w1h, 0, [[1, DK], [1, 1]]))
        b1_sb = singles.tile([DK, 1], F32)
        nc.sync.dma_start(out=b1_sb, in_=bass.AP(b1h, 0, [[1, DK], [1, 1]]))
        b2_sb = singles.tile([DK, 1], F32)
        nc.sync.dma_start(out=b2_sb, in_=bass.AP(b2h, 0, [[1, DK], [1, 1]]))
        om1_sb = singles.tile([DK, 1], F32)
        nc.sync.dma_start(out=om1_sb, in_=bass.AP(om1h, 0, [[0, DK], [1, 1]]))
        om2_sb = singles.tile([DK, 1], F32)
        nc.sync.dma_start(out=om2_sb, in_=bass.AP(om2h, 0, [[0, DK], [1, 1]]))
        w2t_sb = singles.tile([DK, DK], F32)
        nc.sync.dma_start(out=w2t_sb, in_=w2th.ap())
        w3a_sb = singles.tile([DK + 1, CIN * COUT], F32)
        nc.sync.dma_start(out=w3a_sb, in_=w3ah.ap())
        bias_sb = singles.tile([L, COUT], F32)
        nc.sync.dma_start(out=bias_sb, in_=bass.AP(biash, 0, [[0, L], [1, COUT]]))

        # omega-folded layer-1 params
        w1p = singles.tile([DK, 1], F32)
        nc.vector.tensor_mul(w1p, w1_sb, om1_sb)
        b1p = singles.tile([DK, 1], F32)
        nc.vector.tensor_mul(b1p, b1_sb, om1_sb)
        b2p = singles.tile([DK, 1], F32)
        nc.vector.tensor_mul(b2p, b2_sb, om2_sb)

        # ---- SIREN layer 1: h1 = sin(om1*(w1*pos + b1)) ----
        h1 = singles.tile([DK, K], F32)
        nc.scalar.activation(
            out=h1, in_=pos_b, func=mybir.ActivationFunctionType.Sin,
            bias=b1p, scale=w1p,
        )

        # ---- SIREN layer 2: h2 = sin(om2*(w2 @ h1 + b2)); augmented ones row ----
        h2aug = singles.tile([DK + 1, K], F32)
        nc.vector.memset(h2aug[DK:DK + 1, :], 1.0)
        for q in range(5):
            lo = 512 * q
            hi = min(K, lo + 512)
            if lo >= hi:
                break
            z2 = kgps.tile([DK, 512], F32, tag="kg")
            nc.tensor.matmul(
                out=z2[:, :hi - lo], lhsT=w2t_sb, rhs=h1[:, lo:hi],
                start=True, stop=True,
            )
            nc.scalar.activation(
                out=h2aug[0:DK, lo:hi], in_=z2[:, :hi - lo],
                func=mybir.ActivationFunctionType.Sin, bias=b2p, scale=om2_sb,
            )

        # ---- layer 3 -> KT (bf16), KT3[p, i, 32c+o] ----
        KT = singles.tile([L, CIN * 17 * COUT], BF16)
        KT3 = KT.rearrange("p (i k) -> p i k", i=CIN)
        # zero only the tap block (rows >=1 must be 0; row 0 is overwritten)
        nc.vector.memset(KT3[:, :, 16 * COUT:17 * COUT], 0.0)
        for c in range(16):
            for g in range(8):
                kg = kgps.tile([L, 128], F32, tag="kg")
                nc.tensor.matmul(
                    out=kg, lhsT=h2aug[:, 128 * c:128 * (c + 1)],
                    rhs=w3a_sb[:, 128 * g:128 * (g + 1)],
                    start=True, stop=True,
                )
                nc.vector.tensor_copy(
                    KT3[:, 4 * g:4 * (g + 1), 32 * c:32 * (c + 1)],
                    kg.rearrange("p (i o) -> p i o", i=4),
                )
        for g in range(8):
            tap = kgps.tile([1, 128], F32, tag="kg")
            nc.tensor.matmul(
                out=tap, lhsT=h2aug[:, T:T + 1],
                rhs=w3a_sb[:, 128 * g:128 * (g + 1)],
                start=True, stop=True,
            )
            nc.vector.tensor_copy(
                KT3[0:1, 4 * g:4 * (g + 1), 16 * COUT:17 * COUT],
                tap.rearrange("p (i o) -> p i o", i=4),
            )

        # ---- xe: zero-padded bf16 copy of x, staged back to DRAM ----
        x_sb = singles.tile([CIN, bsh, T], F32)
        nc.sync.dma_start(
            out=x_sb, in_=bass.AP(xh, 0, [[T, CIN], [CIN * T, bsh], [1, T]])
        )
        xe_st = singles.tile([CIN, bsh, XE_LEN], BF16)
        nc.vector.memset(xe_st, 0.0)
        nc.vector.tensor_copy(xe_st[:, :, 128:128 + T], x_sb)
        nc.sync.dma_start(
            out=bass.AP(xeh, 0, [[XE_LEN, CIN], [CIN * XE_LEN, bsh], [1, XE_LEN]]),
            in_=xe_st,
        )

        # ---- output accumulators ----
        out_sb = singles.tile([L, bsh, NB, COUT], F32)

        # ---- main loop: Hankel (stationary) x KT-slice (moving) ----
        def conv_body():
            nc.vector.tensor_copy(
                out_sb, bias_sb[:].unsqueeze(1).unsqueeze(1).broadcast_to(
                    [L, bsh, NB, COUT])
            )
            for b in range(bsh):
                for a0 in range(15, 32, 2):
                    # one DMA loads a [128, CIN, 256] tile whose 512B rows
                    # cover Hankel windows a0 and a0+1 (cols 0:128 / 128:256)
                    hank = hankp.tile([L, CIN, 2 * L], BF16, tag="hank")
                    if "dma" not in ablate:
                        half = CIN // 2
                        for eng, ilo in ((nc.sync, 0), (nc.scalar, half)):
                            eng.dma_start(
                                out=hank[:, ilo:ilo + half, :],
                                in_=bass.AP(
                                    xeh,
                                    (b * CIN + ilo) * XE_LEN + L * (a0 - 15),
                                    [[1, L], [XE_LEN, half], [1, 2 * L]],
                                ),
                            )
                    for a in (a0, a0 + 1):
                        if a > 31:
                            continue
                        k = a - a0
                        clo = 0 if a == 15 else a - 15
                        chi = 16 if a == 15 else 17
                        ncols = COUT * (chi - clo)
                        ps = mainps.tile([L, ncols], F32, tag="main")
                        if "mm" not in ablate:
                            for i in range(CIN):
                                nc.tensor.matmul(
                                    out=ps,
                                    lhsT=hank[:, i, L * k:L * (k + 1)],
                                    rhs=KT3[:, i, COUT * clo:COUT * chi],
                                    start=(i == 0), stop=(i == CIN - 1),
                                )
                        else:
                            nc.vector.memset(ps, 0.0)
                        if "add" not in ablate:
                            for j in range(chi - clo):
                                tb = a - (clo + j)
                                nc.vector.tensor_add(
                                    out_sb[:, b, tb, :], out_sb[:, b, tb, :],
                                    ps[:, COUT * j:COUT * (j + 1)],
                                )

        if reps == 1:
            conv_body()
        else:
            with tc.For_i(0, reps, 1):
                conv_body()

        # ---- write out[b,o,t] (dt is partition dim -> contiguous t runs) ----
        for b in range(bsh):
            for tb in range(NB):
                nc.sync.dma_start(
                    out=bass.AP(
                        outh, b * COUT * T + tb * L, [[1, L], [T, COUT]]
                    ),
                    in_=out_sb[:, b, tb, :],
                )

    nc.compile()
    return nc


def _host_prep(inputs):
    """Pure-layout host prep: transposes/reshapes/concats of the weights."""
    w2t = np.ascontiguousarray(np.asarray(inputs["w2"], np.float32).T)
    w3 = np.asarray(inputs["w3"], np.float32)
    b3 = np.asarray(inputs["b3"], np.float32)
    # w3a[m, 32*i + o] = w3[o*CIN + i, m]; w3a[DK, 32*i+o] = b3[o*CIN+i]
    w3r = w3.reshape(COUT, CIN, DK)
    w3a = np.concatenate(
        [w3r.transpose(2, 1, 0).reshape(DK, CIN * COUT),
         b3.reshape(COUT, CIN).T.reshape(1, CIN * COUT)],
        axis=0,
    )
    return {
        "rel_pos": np.ascontiguousarray(np.asarray(inputs["rel_pos"], np.float32)),
        "w1": np.ascontiguousarray(np.asarray(inputs["w1"], np.float32).reshape(DK)),
        "b1": np.ascontiguousarray(np.asarray(inputs["b1"], np.float32)),
        "om1": np.asarray(inputs["omega1"], np.float32).reshape(1).copy(),
        "w2t": w2t,
        "b2": np.ascontiguousarray(np.asarray(inputs["b2"], np.float32)),
        "om2": np.asarray(inputs["omega2"], np.float32).reshape(1).copy(),
        "w3a": np.ascontiguousarray(w3a, dtype=np.float32),
        "bias": np.ascontiguousarray(np.asarray(inputs["bias"], np.float32)),
    }


def kernel(**inputs) -> np.ndarray:
    if "nc" not in _CACHED:
        _CACHED["nc"] = _build()
    nc = _CACHED["nc"]

    x = np.ascontiguousarray(np.asarray(inputs["x"], np.float32))
    shared = _host_prep(inputs)
    in_maps = []
    for c in range(NCORES):
        m = dict(shared)
        m["x"] = np.ascontiguousarray(x[c * BSH:(c + 1) * BSH])
        in_maps.append(m)

    trace = bool(int(os.environ.get("CKCONV_TRACE", "0")))
    res = run_bass_kernel_spmd(nc, in_maps, list(range(NCORES)), trace=trace)
    _CACHED["last_results"] = res
    out = np.concatenate([res.results[c]["out"] for c in range(NCORES)], axis=0)
    return out.astype(np.float32)



# revision 10
# speedup vs baseline: 14.8364x; 14.8364x over previous
"""CKConv (SIREN continuous-kernel causal conv) Trainium2 Bass kernel.

Problem dims (hardcoded): B=32, CIN=32, COUT=32, T=2048, DK=32, K=T+1=2049.

Strategy: data-parallel over batch across 8 NeuronCores (4 samples/core).
Each core:
  1. runs the tiny SIREN kernel-net on-chip (fp32) to generate the conv
     kernel in descending-chunk order KTd[dtau, i, 32m+o] = kern[o, i,
     128*(16-m)+dtau] (bf16), with block m=0 holding the single tap
     kern[o, i, 2048] in row dtau=0; the tap row is also staged to DRAM
     and reloaded as ktap[i, o] for the last output block,
  2. zero-pads its x shard into xe[b,i,:] = [0]*128 ++ x ++ [0]*256 (bf16,
     in DRAM, 128 partitions = (b,i)),
  3. computes the causal conv as Hankel x Toeplitz block matmuls: for each
     input window a in 15..30, the stationary operand is the Hankel tile
     H[b,a,i][p,d] = xe[b,i,128*(a-15)+p+d] (loaded in 4-window "quad"
     tiles -> 1KB DMA rows) and the moving operand is a contiguous slice
     of KTd; because KTd is in descending-chunk order the slice for window
     a lands on PSUM columns 32*tb+o with tb the output time-block, so ALL
     window matmuls of one batch sample accumulate in a single PSUM bank
     (seeded with the output bias via a 1-row matmul) -- no vector-engine
     scatter adds.  The tap-only window a=31 is instead one rank-CIN
     matmul xtap[i,d] @ ktap[i,o] into PSUM columns 480:512,
  4. copies PSUM -> SBUF and DMAs to DRAM in the [b, p, tb, o] layout
     (2KB contiguous runs per partition; the final transpose to [b, o, t]
     is a pure-layout numpy op on host, like the weight reshapes).

The host-side wrapper only reshapes/transposes weights and the output
(pure layout) and concatenates per-core results.
"""
import os
import numpy as np

from contextlib import ExitStack

import concourse.bass as bass
import concourse.tile as tile
from concourse import bacc, mybir
from concourse.bass_utils import run_bass_kernel_spmd

F32 = mybir.dt.float32
BF16 = mybir.dt.bfloat16

B, CIN, COUT, T, DK = 32, 32, 32, 2048, 32
K = T + 1
L = 128
NB = T // L          # 16 time blocks
NCORES = 8
BSH = B // NCORES    # 4 batch samples per core
XE_LEN = 128 + T + 256  # 2432

_CACHED = {}

# conv rounds: 4-window quads sharing one Hankel DMA tile (1KB rows); the
# (15..18) quad goes last, reordered so the final matmul of each PSUM
# chain is full-width (clean stop flag covering the whole bank).
QUADS = [(19, 20, 21, 22), (23, 24, 25, 26), (27, 28, 29, 30), (17, 18, 15, 16)]


def _build(bsh: int = BSH):
    """Build + schedule the per-core Bass program (SPMD, no collectives)."""
    nc = bacc.Bacc(
        "TRN2", target_bir_lowering=False, debug=False, enable_asserts=False
    )

    xh = nc.dram_tensor("x", [bsh, CIN, T], F32, kind="ExternalInput")
    rph = nc.dram_tensor("rel_pos", [K], F32, kind="ExternalInput")
    w1h = nc.dram_tensor("w1", [DK], F32, kind="ExternalInput")
    b1h = nc.dram_tensor("b1", [DK], F32, kind="ExternalInput")
    om1h = nc.dram_tensor("om1", [1], F32, kind="ExternalInput")
    w2th = nc.dram_tensor("w2t", [DK, DK], F32, kind="ExternalInput")
    b2h = nc.dram_tensor("b2", [DK], F32, kind="ExternalInput")
    om2h = nc.dram_tensor("om2", [1], F32, kind="ExternalInput")
    w3ah = nc.dram_tensor("w3a", [DK + 1, CIN * COUT], F32, kind="ExternalInput")
    biash = nc.dram_tensor("bias", [COUT], F32, kind="ExternalInput")
    # out_perm[b, p, tb, o] = out[b, o, 128*tb + p] (host transposes back)
    outh = nc.dram_tensor("out", [bsh, L, NB, COUT], F32, kind="ExternalOutput")

    xeh = nc.dram_tensor("xe", [bsh, CIN, XE_LEN], BF16)    # internal
    ktaph = nc.dram_tensor("ktap", [CIN * COUT], BF16)      # internal

    with tile.TileContext(nc) as tc, ExitStack() as ctx:
        singles = ctx.enter_context(tc.tile_pool(name="singles", bufs=1))
        hankp = ctx.enter_context(tc.tile_pool(name="hankp", bufs=3))
        kgps = ctx.enter_context(tc.tile_pool(name="kgps", bufs=2, space="PSUM"))
        mainps = ctx.enter_context(tc.tile_pool(name="mainps", bufs=1, space="PSUM"))

        # ---- small constants / broadcasts ----
        pos_b = singles.tile([DK, K], F32)
        nc.sync.dma_start(out=pos_b, in_=bass.AP(rph, 0, [[0, DK], [1, K]]))
        w1_sb = singles.tile([DK, 1], F32)
        nc.sync.dma_start(out=w1_sb, in_=bass.AP(w1h, 0, [[1, DK], [1, 1]]))
        b1_sb = singles.tile([DK, 1], F32)
        nc.sync.dma_start(out=b1_sb, in_=bass.AP(b1h, 0, [[1, DK], [1, 1]]))
        b2_sb = singles.tile([DK, 1], F32)
        nc.sync.dma_start(out=b2_sb, in_=bass.AP(b2h, 0, [[1, DK], [1, 1]]))
        om1_sb = singles.tile([DK, 1], F32)
        nc.sync.dma_start(out=om1_sb, in_=bass.AP(om1h, 0, [[0, DK], [1, 1]]))
        om2_sb = singles.tile([DK, 1], F32)
        nc.sync.dma_start(out=om2_sb, in_=bass.AP(om2h, 0, [[0, DK], [1, 1]]))
        w2t_sb = singles.tile([DK, DK], F32)
        nc.sync.dma_start(out=w2t_sb, in_=w2th.ap())
        w3a_sb = singles.tile([DK + 1, CIN * COUT], F32)
        nc.sync.dma_start(out=w3a_sb, in_=w3ah.ap())
        # bias replicated along tb: biasrow[0, 32*tb + o] = bias[o]
        biasrow = singles.tile([1, NB, COUT], F32)
        nc.sync.dma_start(
            out=biasrow, in_=bass.AP(biash, 0, [[0, 1], [0, NB], [1, COUT]])
        )
        biasrow_bf = singles.tile([1, NB * COUT], BF16)
        nc.vector.tensor_copy(biasrow_bf, biasrow.rearrange("p a b -> p (a b)"))
        ones1 = singles.tile([1, L], BF16)
        nc.vector.memset(ones1, 1.0)

        # ---- xe: zero-padded bf16 copy of x, staged back to DRAM ----
        # (emitted first: the sync queue is FIFO and the conv's Hankel DMAs
        # must not queue behind anything that depends on the kernel-net)
        # partitions = (b, i); per-partition free dim = time (contiguous)
        x_sb = singles.tile([L, T], F32)
        nc.sync.dma_start(out=x_sb, in_=bass.AP(xh, 0, [[T, L], [1, T]]))
        xe_st = singles.tile([L, XE_LEN], BF16)
        nc.vector.memset(xe_st[:, 0:L], 0.0)
        nc.vector.memset(xe_st[:, L + T:XE_LEN], 0.0)
        nc.vector.tensor_copy(xe_st[:, L:L + T], x_sb)
        nc.sync.dma_start(
            out=bass.AP(xeh, 0, [[XE_LEN, L], [1, XE_LEN]]), in_=xe_st
        )
        # xtap[i, b, d] = xe[b, i, 2048 + d] = x[b, i, 1920 + d]
        xtap = singles.tile([CIN, bsh, L], BF16, name="xtap")
        for b in range(bsh):
            nc.sync.dma_start(
                out=xtap[:, b, :],
                in_=bass.AP(
                    xeh, (b * CIN) * XE_LEN + 2048, [[XE_LEN, CIN], [1, L]]
                ),
            )

        # omega-folded layer-1 params
        w1p = singles.tile([DK, 1], F32)
        nc.vector.tensor_mul(w1p, w1_sb, om1_sb)
        b1p = singles.tile([DK, 1], F32)
        nc.vector.tensor_mul(b1p, b1_sb, om1_sb)
        b2p = singles.tile([DK, 1], F32)
        nc.vector.tensor_mul(b2p, b2_sb, om2_sb)

        # ---- SIREN layer 1: h1 = sin(om1*(w1*pos + b1)) ----
        h1 = singles.tile([DK, K], F32)
        nc.scalar.activation(
            out=h1, in_=pos_b, func=mybir.ActivationFunctionType.Sin,
            bias=b1p, scale=w1p,
        )

        # ---- SIREN layer 2: h2 = sin(om2*(w2 @ h1 + b2)); augmented ones row ----
        h2aug = singles.tile([DK + 1, K], F32)
        nc.vector.memset(h2aug[DK:DK + 1, :], 1.0)
        for q in range(5):
            lo = 512 * q
            hi = min(K, lo + 512)
            if lo >= hi:
                break
            z2 = kgps.tile([DK, 512], F32, tag="kg")
            nc.tensor.matmul(
                out=z2[:, :hi - lo], lhsT=w2t_sb, rhs=h1[:, lo:hi],
                start=True, stop=True,
            )
            nc.scalar.activation(
                out=h2aug[0:DK, lo:hi], in_=z2[:, :hi - lo],
                func=mybir.ActivationFunctionType.Sin, bias=b2p, scale=om2_sb,
            )

        # ---- layer 3 -> KTd (bf16), descending chunks: block m holds
        # kern[o, i, 128*(16-m) + dtau]; m=0 is the tap block (row 0 only) ----
        KT = singles.tile([L, CIN * 17 * COUT], BF16)
        KTd3 = KT.rearrange("p (i k) -> p i k", i=CIN)
        # zero only the tap block (rows >=1 must be 0; row 0 is overwritten)
        nc.vector.memset(KTd3[:, :, 0:COUT], 0.0)
        for c in range(16):
            m = 16 - c
            for h in range(2):
                kg = kgps.tile([L, 512], F32, tag="kg")
                nc.tensor.matmul(
                    out=kg, lhsT=h2aug[:, 128 * c:128 * (c + 1)],
                    rhs=w3a_sb[:, 512 * h:512 * (h + 1)],
                    start=True, stop=True,
                )
                nc.vector.tensor_copy(
                    KTd3[:, 16 * h:16 * (h + 1), COUT * m:COUT * (m + 1)],
                    kg.rearrange("p (i o) -> p i o", i=16),
                )
        # tap row: kern[o, i, 2048] -> KTd3 block m=0 row 0, and to DRAM for
        # the ktap[i, o] reload used by the rank-CIN tap matmul
        taprow_sb = singles.tile([1, CIN * COUT], BF16)
        for h in range(2):
            tap = kgps.tile([1, 512], F32, tag="kg")
            nc.tensor.matmul(
                out=tap, lhsT=h2aug[:, T:T + 1],
                rhs=w3a_sb[:, 512 * h:512 * (h + 1)],
                start=True, stop=True,
            )
            nc.vector.tensor_copy(
                KTd3[0:1, 16 * h:16 * (h + 1), 0:COUT],
                tap.rearrange("p (i o) -> p i o", i=16),
            )
            nc.vector.tensor_copy(taprow_sb[:, 512 * h:512 * (h + 1)], tap)
        # ktap roundtrip on the SWDGE queue so it never blocks the HWDGE
        # FIFO queues that carry the Hankel loads
        nc.gpsimd.dma_start(out=ktaph.ap(), in_=taprow_sb)
        ktap_sb = singles.tile([CIN, COUT], BF16)
        nc.gpsimd.dma_start(
            out=ktap_sb, in_=bass.AP(ktaph, 0, [[COUT, CIN], [1, COUT]])
        )

        # ---- per-sample PSUM accumulators, seeded with the output bias ----
        ps = [
            mainps.tile([L, NB * COUT], F32, tag=f"ps{b}", name=f"ps{b}")
            for b in range(bsh)
        ]
        for b in range(bsh):
            nc.tensor.matmul(
                out=ps[b], lhsT=ones1, rhs=biasrow_bf, start=True, stop=False
            )
            # tap-only window a=31 (output block tb=15) as one rank-CIN matmul
            nc.tensor.matmul(
                out=ps[b][:, COUT * 15:COUT * 16],
                lhsT=xtap[:, b, :], rhs=ktap_sb, start=False, stop=False,
            )

        # ---- main loop: Hankel (stationary) x KTd-slice (moving) ----
        for quad in QUADS:
            a0 = min(quad)
            for b in range(bsh):
                hank = hankp.tile([L, CIN, 4 * L], BF16, tag="hank")
                half = CIN // 2
                for eng, ilo in ((nc.sync, 0), (nc.scalar, half)):
                    eng.dma_start(
                        out=hank[:, ilo:ilo + half, :],
                        in_=bass.AP(
                            xeh,
                            (b * CIN + ilo) * XE_LEN + L * (a0 - 15),
                            [[1, L], [XE_LEN, half], [1, 4 * L]],
                        ),
                    )
                for a in quad:
                    k = a - a0
                    # KTd blocks m in [mlo, 32-a) land on PSUM col blocks
                    # tb = m + a - 16 (tb in [plo, 16))
                    if a <= 16:
                        mlo, plo = 16 - a, 0
                    else:
                        mlo, plo = 0, a - 16
                    mhi = 32 - a
                    ncols = COUT * (mhi - mlo)
                    for i in range(CIN):
                        nc.tensor.matmul(
                            out=ps[b][:, COUT * plo:COUT * plo + ncols],
                            lhsT=hank[:, i, L * k:L * (k + 1)],
                            rhs=KTd3[:, i, COUT * mlo:COUT * mhi],
                            start=False,
                            stop=(a == 16 and i == CIN - 1),
                        )

        # ---- evacuate PSUM -> SBUF -> DRAM ([b, p, tb, o]; 2KB runs) ----
        for b in range(bsh):
            osb = singles.tile([L, NB * COUT], F32)
            nc.vector.tensor_copy(osb, ps[b])
            nc.sync.dma_start(
                out=bass.AP(
                    outh, b * L * NB * COUT, [[NB * COUT, L], [1, NB * COUT]]
                ),
                in_=osb,
            )

    nc.compile()
    return nc


def _host_prep(inputs):
    """Pure-layout host prep: transposes/reshapes/concats of the weights."""
    w2t = np.ascontiguousarray(np.asarray(inputs["w2"], np.float32).T)
    w3 = np.asarray(inputs["w3"], np.float32)
    b3 = np.asarray(inputs["b3"], np.float32)
    # w3a[m, 32*i + o] = w3[o*CIN + i, m]; w3a[DK, 32*i+o] = b3[o*CIN+i]
    w3r = w3.reshape(COUT, CIN, DK)
    w3a = np.concatenate(
        [w3r.transpose(2, 1, 0).reshape(DK, CIN * COUT),
         b3.reshape(COUT, CIN).T.reshape(1, CIN * COUT)],
        axis=0,
    )
    return {
        "rel_pos": np.ascontiguousarray(np.asarray(inputs["rel_pos"], np.float32)),
        "w1": np.ascontiguousarray(np.asarray(inputs["w1"], np.float32).reshape(DK)),
        "b1": np.ascontiguousarray(np.asarray(inputs["b1"], np.float32)),
        "om1": np.asarray(inputs["omega1"], np.float32).reshape(1).copy(),
        "w2t": w2t,
        "b2": np.ascontiguousarray(np.asarray(inputs["b2"], np.float32)),
        "om2": np.asarray(inputs["omega2"], np.float32).reshape(1).copy(),
        "w3a": np.ascontiguousarray(w3a, dtype=np.float32),
        "bias": np.ascontiguousarray(np.asarray(inputs["bias"], np.float32)),
    }


def kernel(**inputs) -> np.ndarray:
    if "nc" not in _CACHED:
        _CACHED["nc"] = _build()
    nc = _CACHED["nc"]

    x = np.ascontiguousarray(np.asarray(inputs["x"], np.float32))
    shared = _host_prep(inputs)
    in_maps = []
    for c in range(NCORES):
        m = dict(shared)
        m["x"] = np.ascontiguousarray(x[c * BSH:(c + 1) * BSH])
        in_maps.append(m)

    trace = bool(int(os.environ.get("CKCONV_TRACE", "0")))
    res = run_bass_kernel_spmd(nc, in_maps, list(range(NCORES)), trace=trace)
    _CACHED["last_results"] = res
    # out_perm[b, p, tb, o] -> out[b, o, 128*tb + p] (pure layout)
    outs = []
    for c in range(NCORES):
        op = res.results[c]["out"]
        outs.append(op.transpose(0, 3, 2, 1).reshape(BSH, COUT, T))
    return np.concatenate(outs, axis=0).astype(np.float32)
